# revision 24
# baseline (speedup 1.0000x reference)
"""Trainium2 Bass kernel for DensePose sparse GN head (segment_reduce).

out = relu((x - mu[seg]) * rstd[seg]) * sigmoid(conv1d(segmean(relu(xn))))[seg]

Host pre-sorts points by segment id and deals them evenly across the 8
cores so every (core, segment) run has identical length LP (padded with
duplicated points; counts become the compile-time constant 8*LP).  Data
is f16, channel-major, two point-halves packed on 128 partitions:

    x_dev[half*64 + ch, s*LP2 + t] = x[pt(core, s, 2*t + half), ch]

Every segment is a contiguous column range: segment sums are free-dim
reductions (DVE tensor_tensor_reduce / Act accumulate), normalize+ECA
scaling is per-partition scale/bias.  The ECA conv over channels
(= partitions) is a tiny tridiagonal-band 64x64 PE matmul.

The computation is separable per segment, so segments are processed in
NBLK pipelined blocks: pass1(b) -> AllReduce1(b) -> pass2(b) ->
AllReduce2(b) -> pass3(b), with blocks interleaved so the collectives
hide behind other blocks' streaming.

Identities (w > 0, rstd > 0):
  out = relu(x*S3 + B3),  S3 = rstd*w,  B3 = -mu*rstd*w
  segsum(relu(x - mu)) = segsum(max(x, mu)) - cnt*mu
"""

import sys

for _p in ("/opt/trn_rl_repo",):
    if _p not in sys.path:
        sys.path.append(_p)

import numpy as np

import concourse.bass as bass
import concourse.bacc as bacc
import concourse.mybir as mybir
import concourse.tile as tile
from concourse.bass_utils import run_bass_kernel_spmd

F32 = mybir.dt.float32
F16 = mybir.dt.float16
ALU = mybir.AluOpType
ACTF = mybir.ActivationFunctionType

C = 64
NSEG = 64
EPS = 1e-5
NCORES = 8
GSEG = 4           # segments per DMA group
NBLK = 4           # pipeline blocks
BSEG = NSEG // NBLK   # segments per block (16)

# engine split knobs (per 16-seg block); sum(x^2) always runs on Act
P1_DVE = 15        # segs whose sum(x) runs on DVE (rest on Act copy+accum)
P2_DVE = 5         # segs whose relu-sum runs on DVE (rest on Act relu+accum)
P3_DVE = 16        # segs whose output op runs on DVE (rest on Act relu)
NRES = 6           # trailing DMA groups kept resident in SBUF (of 16)


def _consts(nc):
    ii = np.vstack([np.eye(64, dtype=np.float32)] * 2)          # [128, 64]
    band3 = np.zeros((64, 192), np.float32)
    for j, off in enumerate((-1, 0, 1)):
        for c in range(64):
            cp = c + off
            if 0 <= cp < 64:
                band3[cp, 64 * j + c] = 1.0
    ones_row = np.ones((1, 64), np.float32)
    c = {}
    c["ii"] = nc.inline_tensor(np.ascontiguousarray(ii), name="ii_f")
    c["band3"] = nc.inline_tensor(np.ascontiguousarray(band3), name="band3_f")
    c["ones_row"] = nc.inline_tensor(ones_row, name="ones_row_f")
    return c


def build_nc(lp2):
    ncols = NSEG * lp2
    glen = GSEG * lp2
    blen = BSEG * lp2
    gpb = BSEG // GSEG           # DMA groups per block
    rcnt = 1.0 / float(NCORES * 2 * lp2)     # 1 / (8 * LP)

    nc = bacc.Bacc("TRN2", target_bir_lowering=False, debug=False,
                   num_devices=NCORES)
    x_ext = nc.declare_dram_parameter("xT", [128, ncols], F16, isOutput=False)
    eca_ext = nc.declare_dram_parameter("eca_weight", [1, 3], F32,
                                        isOutput=False)
    out_ext = nc.declare_dram_parameter("out", [128, ncols], F16,
                                        isOutput=True)
    cst = _consts(nc)
    rg = [list(range(NCORES))]

    with tile.TileContext(nc) as tc:
        with (
            tc.tile_pool(name="const", bufs=1) as constp,
            tc.tile_pool(name="dram", bufs=1, space="DRAM") as dramp,
            tc.tile_pool(name="small", bufs=1) as smallp,
            tc.tile_pool(name="xin", bufs=4) as xinp,
            tc.tile_pool(name="res", bufs=1) as resp,
            tc.tile_pool(name="outp", bufs=2) as outp,
            tc.tile_pool(name="ps", bufs=2, space="PSUM") as psp,
        ):
            # ---- constants ----
            ii_sb = constp.tile([128, 64], F32)
            band3_sb = constp.tile([64, 192], F32)
            ones_sb = constp.tile([1, 64], F32)
            eca_sb = constp.tile([1, 3], F32)
            nc.scalar.dma_start(ii_sb[:], cst["ii"][:, :])
            nc.scalar.dma_start(band3_sb[:], cst["band3"][:, :])
            nc.scalar.dma_start(ones_sb[:], cst["ones_row"][:, :])
            nc.scalar.dma_start(eca_sb[:], eca_ext[:, :])

            # ---- collective bounce buffers (per block) ----
            ar1_in = [dramp.tile([64, 2 * BSEG], F32, name=f"ar1_in{b}")
                      for b in range(NBLK)]
            ar1_out = [dramp.tile([64, 2 * BSEG], F32, addr_space="Shared",
                                  name=f"ar1_out{b}") for b in range(NBLK)]
            ar2_in = [dramp.tile([64, BSEG], F32, name=f"ar2_in{b}")
                      for b in range(NBLK)]
            ar2_out = [dramp.tile([64, BSEG], F32, addr_space="Shared",
                                  name=f"ar2_out{b}") for b in range(NBLK)]

            # ---- persistent small tensors ----
            s1_T = smallp.tile([128, 64], F32)    # sum(x) per (half*ch, seg)
            s2_T = smallp.tile([128, 64], F32)    # sum(x^2)
            msC_T = smallp.tile([128, 64], F32)   # relu-sum (corrected)
            g1_sb = smallp.tile([64, 2 * 64], F32)
            g2_sb = smallp.tile([64, 64], F32)
            fold_sb = smallp.tile([64, 2 * 64], F32)
            fold2_sb = smallp.tile([64, 64], F32)
            mu_T = smallp.tile([64, 64], F32)     # [ch, seg]
            es2 = smallp.tile([64, 64], F32)
            var_T = smallp.tile([64, 64], F32)
            sd_T = smallp.tile([64, 64], F32)
            rstd_T = smallp.tile([64, 64], F32)
            m_T = smallp.tile([64, 64], F32)
            w_T = smallp.tile([64, 64], F32)
            s3_T = smallp.tile([64, 64], F32)
            b3_T = smallp.tile([64, 64], F32)
            band_sb = smallp.tile([64, 64], F32)
            eca_b = smallp.tile([64, 3], F32)
            mu_pack = smallp.tile([128, 64], F32)
            nmu_pack = smallp.tile([128, 64], F32)
            scale_pack = smallp.tile([128, 64], F32)
            bias_pack = smallp.tile([128, 64], F32)
            eps_col = smallp.tile([64, 1], F32)
            nc.vector.memset(eps_col[:], float(EPS))
            # write-only scratch (one per engine; WAW on same queue is free)
            sa_scr = smallp.tile([128, lp2], F16)
            sd_scr = smallp.tile([128, lp2], F16)

            G_RES = NSEG // GSEG - NRES     # groups >= G_RES stay resident
            res_tiles = {}
            # prefetch resident groups at t=0 on the gpsimd DMA queue so
            # they never contend with the in-order stream-load queue
            for g in range(G_RES, NSEG // GSEG):
                xt = resp.tile([128, glen], F16, tag=f"res{g}",
                               name=f"res{g}")
                res_tiles[g] = xt
                nc.gpsimd.dma_start(xt[:], x_ext[:, g * glen:(g + 1) * glen])

            def load_group(g):
                """Stream group g into a rotating tile (sync queue)."""
                if g >= G_RES:
                    return res_tiles[g]
                xt = xinp.tile([128, glen], F16, tag="x", name="xt")
                nc.sync.dma_start(xt[:], x_ext[:, g * glen:(g + 1) * glen])
                return xt

            get_group = load_group

            # ---- ECA weight broadcast + band matrix (input-only deps) ----
            eca_ps = psp.tile([64, 3], F32, tag="eca")
            nc.tensor.matmul(eca_ps[:], ones_sb[:], eca_sb[:],
                             start=True, stop=True, skip_group_check=True)
            nc.scalar.copy(eca_b[:], eca_ps[:])
            nc.vector.tensor_scalar(band_sb[:], band3_sb[:, 0:64],
                                    eca_b[:, 0:1], None, ALU.mult)
            nc.vector.scalar_tensor_tensor(band_sb[:], band3_sb[:, 64:128],
                                           eca_b[:, 1:2], band_sb[:],
                                           ALU.mult, ALU.add)
            nc.vector.scalar_tensor_tensor(band_sb[:], band3_sb[:, 128:192],
                                           eca_b[:, 2:3], band_sb[:],
                                           ALU.mult, ALU.add)

            def S1(b):
                """pass 1 streaming for block b: sum(x), sum(x^2)."""
                for gi in range(gpb):
                    g = b * gpb + gi
                    xt = load_group(g)
                    for k in range(GSEG):
                        s = g * GSEG + k
                        sk = s - b * BSEG      # index within block [0,16)
                        lo, hi = k * lp2, (k + 1) * lp2
                        if sk < P1_DVE:          # sum(x) on DVE
                            nc.vector.tensor_scalar(
                                sd_scr[:], xt[:, lo:hi], 0.0, None,
                                ALU.add, op1=ALU.add,
                                accum_out=s1_T[:, s:s + 1])
                        else:                    # sum(x) on Act (copy)
                            nc.scalar.activation(
                                sa_scr[:], xt[:, lo:hi], ACTF.Copy,
                                accum_out=s1_T[:, s:s + 1])
                        # sum(x^2) on Act
                        nc.scalar.activation(
                            sa_scr[:], xt[:, lo:hi], ACTF.Square,
                            accum_out=s2_T[:, s:s + 1])

            def F1(b):
                """fold halves + AllReduce #1 for block b."""
                c0, c1 = b * BSEG, (b + 1) * BSEG
                fps = psp.tile([64, 2 * BSEG], F32, tag="fold1")
                nc.tensor.matmul(fps[:, 0:BSEG], ii_sb[:], s1_T[:, c0:c1],
                                 start=True, stop=True, skip_group_check=True)
                nc.tensor.matmul(fps[:, BSEG:2 * BSEG], ii_sb[:],
                                 s2_T[:, c0:c1],
                                 start=True, stop=True, skip_group_check=True)
                nc.scalar.copy(fold_sb[:, 2 * c0:2 * c1], fps[:])
                nc.gpsimd.dma_start(ar1_in[b][:], fold_sb[:, 2 * c0:2 * c1])
                nc.gpsimd.collective_compute(
                    "AllReduce", ALU.add, replica_groups=rg,
                    ins=[ar1_in[b][:]], outs=[ar1_out[b][:]])
                nc.gpsimd.dma_start(g1_sb[:, 2 * c0:2 * c1], ar1_out[b][:])

            def D1(b):
                """derive mu, rstd, packs for block b."""
                c0, c1 = b * BSEG, (b + 1) * BSEG
                ga = g1_sb[:, 2 * c0:2 * c0 + BSEG]
                gb = g1_sb[:, 2 * c0 + BSEG:2 * c1]
                nc.vector.tensor_scalar(mu_T[:, c0:c1], ga, rcnt, None,
                                        ALU.mult)
                nc.vector.tensor_scalar(es2[:, c0:c1], gb, rcnt, None,
                                        ALU.mult)
                nc.vector.tensor_tensor(var_T[:, c0:c1], mu_T[:, c0:c1],
                                        mu_T[:, c0:c1], ALU.mult)
                nc.vector.scalar_tensor_tensor(var_T[:, c0:c1],
                                               var_T[:, c0:c1], -1.0,
                                               es2[:, c0:c1],
                                               ALU.mult, ALU.add)
                nc.scalar.activation(sd_T[:, c0:c1], var_T[:, c0:c1],
                                     ACTF.Sqrt, bias=eps_col[:])
                nc.vector.reciprocal(rstd_T[:, c0:c1], sd_T[:, c0:c1])
                nc.vector.tensor_copy(mu_pack[0:64, c0:c1], mu_T[:, c0:c1])
                nc.scalar.dma_start(mu_pack[64:128, c0:c1],
                                  mu_pack[0:64, c0:c1])
                nc.vector.tensor_scalar(nmu_pack[0:64, c0:c1],
                                        mu_T[:, c0:c1], -1.0, None, ALU.mult)
                nc.scalar.dma_start(nmu_pack[64:128, c0:c1],
                                  nmu_pack[0:64, c0:c1])

            def S2(b):
                """pass 2 streaming for block b: sum(relu(x - mu))."""
                for gi in range(gpb):
                    g = b * gpb + gi
                    xt = get_group(g)
                    for k in range(GSEG):
                        s = g * GSEG + k
                        sk = s - b * BSEG
                        lo, hi = k * lp2, (k + 1) * lp2
                        if sk < P2_DVE:
                            # sum(max(x, mu)); corrected by -LP2*mu in C2F2
                            nc.vector.tensor_scalar(
                                sd_scr[:], xt[:, lo:hi],
                                mu_pack[:, s:s + 1], None,
                                ALU.max, op1=ALU.add,
                                accum_out=msC_T[:, s:s + 1])
                        else:
                            # relu(x - mu) summed directly on Act
                            nc.scalar.activation(
                                sa_scr[:], xt[:, lo:hi], ACTF.Relu,
                                bias=nmu_pack[:, s:s + 1],
                                accum_out=msC_T[:, s:s + 1])

            def C2F2(b):
                """correct DVE relu-sums, fold, AllReduce #2 for block b."""
                c0, c1 = b * BSEG, (b + 1) * BSEG
                nc.vector.scalar_tensor_tensor(
                    msC_T[:, c0:c0 + P2_DVE], mu_pack[:, c0:c0 + P2_DVE],
                    float(-lp2), msC_T[:, c0:c0 + P2_DVE],
                    ALU.mult, ALU.add)
                fps = psp.tile([64, BSEG], F32, tag="fold2")
                nc.tensor.matmul(fps[:], ii_sb[:], msC_T[:, c0:c1],
                                 start=True, stop=True, skip_group_check=True)
                nc.scalar.copy(fold2_sb[:, c0:c1], fps[:])
                nc.gpsimd.dma_start(ar2_in[b][:], fold2_sb[:, c0:c1])
                nc.gpsimd.collective_compute(
                    "AllReduce", ALU.add, replica_groups=rg,
                    ins=[ar2_in[b][:]], outs=[ar2_out[b][:]])
                nc.gpsimd.dma_start(g2_sb[:, c0:c1], ar2_out[b][:])

            def D2(b):
                """m, ECA conv, sigmoid, S3/B3 packs for block b."""
                c0, c1 = b * BSEG, (b + 1) * BSEG
                nc.vector.tensor_scalar(m_T[:, c0:c1], g2_sb[:, c0:c1],
                                        rcnt, None, ALU.mult)
                nc.vector.tensor_tensor(m_T[:, c0:c1], m_T[:, c0:c1],
                                        rstd_T[:, c0:c1], ALU.mult)
                cps = psp.tile([64, BSEG], F32, tag="conv")
                nc.tensor.matmul(cps[:], band_sb[:], m_T[:, c0:c1],
                                 start=True, stop=True, skip_group_check=True)
                nc.scalar.activation(w_T[:, c0:c1], cps[:], ACTF.Sigmoid)
                nc.vector.tensor_tensor(s3_T[:, c0:c1], rstd_T[:, c0:c1],
                                        w_T[:, c0:c1], ALU.mult)
                nc.vector.scalar_tensor_tensor(b3_T[:, c0:c1], mu_T[:, c0:c1],
                                               -1.0, s3_T[:, c0:c1],
                                               ALU.mult, ALU.mult)
                nc.vector.tensor_copy(scale_pack[0:64, c0:c1], s3_T[:, c0:c1])
                nc.scalar.dma_start(scale_pack[64:128, c0:c1],
                                  scale_pack[0:64, c0:c1])
                nc.vector.tensor_copy(bias_pack[0:64, c0:c1], b3_T[:, c0:c1])
                nc.scalar.dma_start(bias_pack[64:128, c0:c1],
                                  bias_pack[0:64, c0:c1])

            def S3(b):
                """pass 3 streaming for block b: out = relu(x*S3 + B3)."""
                for gi in range(gpb):
                    g = b * gpb + gi
                    xt = get_group(g)
                    ot = outp.tile([128, glen], F16, tag="o")
                    for k in range(GSEG):
                        s = g * GSEG + k
                        sk = s - b * BSEG
                        lo, hi = k * lp2, (k + 1) * lp2
                        if sk < P3_DVE:
                            nc.vector.tensor_scalar(
                                ot[:, lo:hi], xt[:, lo:hi],
                                scale_pack[:, s:s + 1],
                                bias_pack[:, s:s + 1], ALU.mult, op1=ALU.add)
                            nc.vector.tensor_scalar(
                                ot[:, lo:hi], ot[:, lo:hi], 0.0, None,
                                ALU.max)
                        else:
                            nc.scalar.activation(
                                ot[:, lo:hi], xt[:, lo:hi], ACTF.Relu,
                                bias=bias_pack[:, s:s + 1],
                                scale=scale_pack[:, s:s + 1])
                    nc.scalar.dma_start(out_ext[:, g * glen:(g + 1) * glen],
                                      ot[:])

            # ---- pipelined schedule over blocks ----
            S1(0); F1(0)
            S1(1); F1(1)
            S1(2); F1(2)
            S1(3); F1(3)
            D1(0); S2(0); C2F2(0)
            D1(1); S2(1); C2F2(1)
            D2(0); S3(0)
            D1(2); S2(2); C2F2(2)
            D2(1); S3(1)
            D1(3); S2(3); C2F2(3)
            D2(2); S3(2)
            D2(3); S3(3)

    nc.compile()
    return nc


_cache = {}


def _get_nc(lp2):
    if lp2 not in _cache:
        _cache[lp2] = build_nc(lp2)
    return _cache[lp2]


last_result = None


def _install_ntff_hook():
    """Provide antenv.axon_hooks (missing in this image) so
    run_bass_kernel_spmd(trace=True) can reach the axon NTFF profiler."""
    import types

    try:
        from antenv.axon_hooks import get_axon_ntff_profile_hook  # noqa: F401
        return
    except ImportError:
        pass
    if "/root/.axon_site" not in sys.path:
        sys.path.insert(0, "/root/.axon_site")
    from trn_agent_boot.trn_boot import _ntff_profile_via_ctypes
    hook = _ntff_profile_via_ctypes("/opt/axon/libaxon_pjrt.so")
    try:
        import antenv
    except ImportError:
        antenv = types.ModuleType("antenv")
        sys.modules["antenv"] = antenv
    mod = types.ModuleType("antenv.axon_hooks")
    mod.get_axon_ntff_profile_hook = lambda: hook
    mod.set_axon_ntff_profile_hook = lambda h: None
    sys.modules["antenv.axon_hooks"] = mod
    antenv.axon_hooks = mod
    import concourse.bass_utils as _bu
    _bu.upload_artifacts = lambda d: "local://" + str(d)


def _prep_inputs(x, idx, eca):
    """Sort by segment, deal evenly over cores, pad each (core, seg) run
    to the common even length LP with duplicated points."""
    order = np.argsort(idx, kind="stable")
    counts = np.bincount(idx, minlength=NSEG).astype(np.int64)
    starts = np.zeros(NSEG + 1, np.int64)
    starts[1:] = np.cumsum(counts)

    q, r = np.divmod(counts, NCORES)
    maxchunk = int((q + (r > 0).astype(np.int64)).max())
    lp = max(2, ((maxchunk + 1) // 2) * 2)       # even
    lp2 = lp // 2

    grids = []
    in_maps = []
    for kcore in range(NCORES):
        grid = np.empty((NSEG, lp), np.int64)
        for s in range(NSEG):
            n_s = counts[s]
            run = order[starts[s]:starts[s] + n_s]
            qq, rr = divmod(int(n_s), NCORES)
            a = kcore * qq + min(kcore, rr)
            b = a + qq + (1 if kcore < rr else 0)
            chunk = run[a:b]
            assert chunk.size > 0, f"empty (core,seg)=({kcore},{s})"
            grid[s] = np.resize(chunk, lp)
        grids.append(grid)
        xg = x[grid.reshape(-1)].reshape(NSEG, lp2, 2, C)
        dev = np.ascontiguousarray(
            xg.transpose(2, 3, 0, 1).reshape(128, NSEG * lp2),
            dtype=np.float16)
        in_maps.append({"xT": dev, "eca_weight": eca})
    return in_maps, grids, lp2


def kernel(features, ins_indices_batch, eca_weight, _trace=False):
    global last_result
    x = np.asarray(features, np.float32)
    idx = np.asarray(ins_indices_batch, np.int32)
    eca = np.asarray(eca_weight, np.float32).reshape(1, 3)
    n = x.shape[0]

    in_maps, grids, lp2 = _prep_inputs(x, idx, eca)
    nc = _get_nc(lp2)

    if _trace:
        _install_ntff_hook()
    try:
        res = run_bass_kernel_spmd(nc, in_maps, core_ids=list(range(NCORES)),
                                   trace=_trace)
    except Exception:
        if not _trace:
            raise
        import traceback
        traceback.print_exc()
        print("traced run failed; falling back to untraced", flush=True)
        res = run_bass_kernel_spmd(nc, in_maps, core_ids=list(range(NCORES)))
    last_result = res

    out = np.empty((n, C), np.float32)
    lp = 2 * lp2
    for kcore in range(NCORES):
        od = res.results[kcore]["out"]            # [128, NSEG*lp2] f16
        vals = od.reshape(2, C, NSEG, lp2).transpose(2, 3, 0, 1)
        out[grids[kcore].reshape(-1)] = vals.reshape(NSEG * lp, C)
    return out


if __name__ == "__main__":
    rng = np.random.default_rng(0)
    n_test = 200_000
    x = rng.standard_normal((n_test, C), dtype=np.float32)
    ii = rng.integers(0, NSEG, n_test).astype(np.int32)
    k = (rng.standard_normal((1, 1, 3)) * 0.1).astype(np.float32)
    out = kernel(x, ii, k)

    seg = ii
    cnt = np.maximum(np.bincount(seg, minlength=NSEG), 1).astype(np.float64)
    s = np.zeros((NSEG, C)); np.add.at(s, seg, x.astype(np.float64))
    s2 = np.zeros((NSEG, C)); np.add.at(s2, seg, x.astype(np.float64) ** 2)
    mu = s / cnt[:, None]
    var = s2 / cnt[:, None] - mu ** 2
    xn = (x - mu[seg]) / np.sqrt(var[seg] + EPS)
    xr = np.maximum(xn, 0)
    m = np.zeros((NSEG, C)); np.add.at(m, seg, xr)
    m = m / cnt[:, None]
    kf = k.reshape(3)
    mp = np.pad(m, ((0, 0), (1, 1)))
    conv = kf[0] * mp[:, 0:64] + kf[1] * mp[:, 1:65] + kf[2] * mp[:, 2:66]
    w = 1.0 / (1.0 + np.exp(-conv))
    exp = xr * w[seg]
    err = np.linalg.norm(out - exp) / np.linalg.norm(exp)
    print("out", out.shape, out.dtype, "rel_err", err)


# revision 26
# speedup vs baseline: 1.0289x; 1.0289x over previous
"""Trainium2 Bass kernel for DensePose sparse GN head (segment_reduce).

out = relu((x - mu[seg]) * rstd[seg]) * sigmoid(conv1d(segmean(relu(xn))))[seg]

Host pre-sorts points by segment id and deals them evenly across the 8
cores so every (core, segment) run has identical length LP (padded with
duplicated points; counts become the compile-time constant 8*LP).  Data
is f16, channel-major, two point-halves packed on 128 partitions:

    x_dev[half*64 + ch, s*LP2 + t] = x[pt(core, s, 2*t + half), ch]

Every segment is a contiguous column range: segment sums are free-dim
reductions (DVE tensor_tensor_reduce / Act accumulate), normalize+ECA
scaling is per-partition scale/bias.  The ECA conv over channels
(= partitions) is a tiny tridiagonal-band 64x64 PE matmul.

The computation is separable per segment, so segments are processed in
NBLK pipelined blocks: pass1(b) -> AllReduce1(b) -> pass2(b) ->
AllReduce2(b) -> pass3(b), with blocks interleaved so the collectives
hide behind other blocks' streaming.

Identities (w > 0, rstd > 0):
  out = relu(x*S3 + B3),  S3 = rstd*w,  B3 = -mu*rstd*w
  segsum(relu(x - mu)) = segsum(max(x, mu)) - cnt*mu
"""

import sys

for _p in ("/opt/trn_rl_repo",):
    if _p not in sys.path:
        sys.path.append(_p)

import numpy as np

import concourse.bass as bass
import concourse.bacc as bacc
import concourse.mybir as mybir
import concourse.tile as tile
from concourse.bass_utils import run_bass_kernel_spmd

F32 = mybir.dt.float32
F16 = mybir.dt.float16
ALU = mybir.AluOpType
ACTF = mybir.ActivationFunctionType

C = 64
NSEG = 64
EPS = 1e-5
NCORES = 8
GSEG = 4           # segments per DMA group
NBLK = 4           # pipeline blocks
BSEG = NSEG // NBLK   # segments per block (16)

# engine split knobs (per 16-seg block); sum(x^2) always runs on Act
P1_DVE = 15        # segs whose sum(x) runs on DVE (rest on Act copy+accum)
P2_DVE = 5         # segs whose relu-sum runs on DVE (rest on Act relu+accum)
P3_DVE = 16        # segs whose output op runs on DVE (rest on Act relu)
NRES = 6           # trailing DMA groups kept resident in SBUF (of 16)


def _consts(nc):
    ii = np.vstack([np.eye(64, dtype=np.float32)] * 2)          # [128, 64]
    band3 = np.zeros((64, 192), np.float32)
    for j, off in enumerate((-1, 0, 1)):
        for c in range(64):
            cp = c + off
            if 0 <= cp < 64:
                band3[cp, 64 * j + c] = 1.0
    ones_row = np.ones((1, 64), np.float32)
    c = {}
    c["ii"] = nc.inline_tensor(np.ascontiguousarray(ii), name="ii_f")
    c["band3"] = nc.inline_tensor(np.ascontiguousarray(band3), name="band3_f")
    c["ones_row"] = nc.inline_tensor(ones_row, name="ones_row_f")
    return c


def build_nc(lp2):
    ncols = NSEG * lp2
    glen = GSEG * lp2
    blen = BSEG * lp2
    gpb = BSEG // GSEG           # DMA groups per block
    rcnt = 1.0 / float(NCORES * 2 * lp2)     # 1 / (8 * LP)

    nc = bacc.Bacc("TRN2", target_bir_lowering=False, debug=False,
                   num_devices=NCORES)
    x_ext = nc.declare_dram_parameter("xT", [128, ncols], F16, isOutput=False)
    eca_ext = nc.declare_dram_parameter("eca_weight", [1, 3], F32,
                                        isOutput=False)
    out_ext = nc.declare_dram_parameter("out", [128, ncols], F16,
                                        isOutput=True)
    cst = _consts(nc)
    rg = [list(range(NCORES))]

    with tile.TileContext(nc) as tc:
        with (
            tc.tile_pool(name="const", bufs=1) as constp,
            tc.tile_pool(name="dram", bufs=1, space="DRAM") as dramp,
            tc.tile_pool(name="small", bufs=1) as smallp,
            tc.tile_pool(name="xin", bufs=4) as xinp,
            tc.tile_pool(name="res", bufs=1) as resp,
            tc.tile_pool(name="outp", bufs=2) as outp,
            tc.tile_pool(name="ps", bufs=2, space="PSUM") as psp,
        ):
            # ---- constants ----
            ii_sb = constp.tile([128, 64], F32)
            band3_sb = constp.tile([64, 192], F32)
            ones_sb = constp.tile([1, 64], F32)
            eca_sb = constp.tile([1, 3], F32)
            nc.scalar.dma_start(ii_sb[:], cst["ii"][:, :])
            nc.scalar.dma_start(band3_sb[:], cst["band3"][:, :])
            nc.scalar.dma_start(ones_sb[:], cst["ones_row"][:, :])
            nc.scalar.dma_start(eca_sb[:], eca_ext[:, :])

            # ---- collective bounce buffers (per block) ----
            ar1_in = [dramp.tile([64, 2 * BSEG], F32, name=f"ar1_in{b}")
                      for b in range(NBLK)]
            ar1_out = [dramp.tile([64, 2 * BSEG], F32, addr_space="Shared",
                                  name=f"ar1_out{b}") for b in range(NBLK)]
            ar2_in = [dramp.tile([64, BSEG], F32, name=f"ar2_in{b}")
                      for b in range(NBLK)]
            ar2_out = [dramp.tile([64, BSEG], F32, addr_space="Shared",
                                  name=f"ar2_out{b}") for b in range(NBLK)]

            # ---- persistent small tensors ----
            s1_T = smallp.tile([128, 64], F32)    # sum(x) per (half*ch, seg)
            s2_T = smallp.tile([128, 64], F32)    # sum(x^2)
            msC_T = smallp.tile([128, 64], F32)   # relu-sum (corrected)
            g1_sb = smallp.tile([64, 2 * 64], F32)
            g2_sb = smallp.tile([64, 64], F32)
            fold_sb = smallp.tile([64, 2 * 64], F32)
            fold2_sb = smallp.tile([64, 64], F32)
            mu_T = smallp.tile([64, 64], F32)     # [ch, seg]
            es2 = smallp.tile([64, 64], F32)
            var_T = smallp.tile([64, 64], F32)
            sd_T = smallp.tile([64, 64], F32)
            rstd_T = smallp.tile([64, 64], F32)
            m_T = smallp.tile([64, 64], F32)
            w_T = smallp.tile([64, 64], F32)
            s3_T = smallp.tile([64, 64], F32)
            b3_T = smallp.tile([64, 64], F32)
            band_sb = smallp.tile([64, 64], F32)
            eca_b = smallp.tile([64, 3], F32)
            mu_pack = smallp.tile([128, 64], F32)
            nmu_pack = smallp.tile([128, 64], F32)
            scale_pack = smallp.tile([128, 64], F32)
            bias_pack = smallp.tile([128, 64], F32)
            eps_col = smallp.tile([64, 1], F32)
            nc.vector.memset(eps_col[:], float(EPS))
            # write-only scratch (one per engine; WAW on same queue is free)
            sa_scr = smallp.tile([128, lp2], F16)
            sd_scr = smallp.tile([128, lp2], F16)

            G_RES = NSEG // GSEG - NRES     # groups >= G_RES stay resident
            res_tiles = {}
            # prefetch resident groups at t=0 on the gpsimd DMA queue so
            # they never contend with the in-order stream-load queue
            for g in range(G_RES, NSEG // GSEG):
                xt = resp.tile([128, glen], F16, tag=f"res{g}",
                               name=f"res{g}")
                res_tiles[g] = xt
                nc.gpsimd.dma_start(xt[:], x_ext[:, g * glen:(g + 1) * glen])

            def load_group(g):
                """Stream group g into a rotating tile (sync queue)."""
                if g >= G_RES:
                    return res_tiles[g]
                xt = xinp.tile([128, glen], F16, tag="x", name="xt")
                nc.sync.dma_start(xt[:], x_ext[:, g * glen:(g + 1) * glen])
                return xt

            get_group = load_group

            # ---- ECA weight broadcast + band matrix (input-only deps) ----
            eca_ps = psp.tile([64, 3], F32, tag="eca")
            nc.tensor.matmul(eca_ps[:], ones_sb[:], eca_sb[:],
                             start=True, stop=True, skip_group_check=True)
            nc.scalar.copy(eca_b[:], eca_ps[:])
            nc.vector.tensor_scalar(band_sb[:], band3_sb[:, 0:64],
                                    eca_b[:, 0:1], None, ALU.mult)
            nc.vector.scalar_tensor_tensor(band_sb[:], band3_sb[:, 64:128],
                                           eca_b[:, 1:2], band_sb[:],
                                           ALU.mult, ALU.add)
            nc.vector.scalar_tensor_tensor(band_sb[:], band3_sb[:, 128:192],
                                           eca_b[:, 2:3], band_sb[:],
                                           ALU.mult, ALU.add)

            def S1(b):
                """pass 1 streaming for block b: sum(x), sum(x^2)."""
                for gi in range(gpb):
                    g = b * gpb + gi
                    xt = load_group(g)
                    for k in range(GSEG):
                        s = g * GSEG + k
                        sk = s - b * BSEG      # index within block [0,16)
                        lo, hi = k * lp2, (k + 1) * lp2
                        if sk < P1_DVE:          # sum(x) on DVE
                            nc.vector.tensor_scalar(
                                sd_scr[:], xt[:, lo:hi], 0.0, None,
                                ALU.add, op1=ALU.add,
                                accum_out=s1_T[:, s:s + 1])
                        else:                    # sum(x) on Act (copy)
                            nc.scalar.activation(
                                sa_scr[:], xt[:, lo:hi], ACTF.Copy,
                                accum_out=s1_T[:, s:s + 1])
                        # sum(x^2) on Act
                        nc.scalar.activation(
                            sa_scr[:], xt[:, lo:hi], ACTF.Square,
                            accum_out=s2_T[:, s:s + 1])

            def F1(b):
                """fold halves + AllReduce #1 for block b."""
                c0, c1 = b * BSEG, (b + 1) * BSEG
                fps = psp.tile([64, 2 * BSEG], F32, tag="fold1")
                nc.tensor.matmul(fps[:, 0:BSEG], ii_sb[:], s1_T[:, c0:c1],
                                 start=True, stop=True, skip_group_check=True)
                nc.tensor.matmul(fps[:, BSEG:2 * BSEG], ii_sb[:],
                                 s2_T[:, c0:c1],
                                 start=True, stop=True, skip_group_check=True)
                nc.scalar.copy(fold_sb[:, 2 * c0:2 * c1], fps[:])
                nc.gpsimd.dma_start(ar1_in[b][:], fold_sb[:, 2 * c0:2 * c1])
                nc.gpsimd.collective_compute(
                    "AllReduce", ALU.add, replica_groups=rg,
                    ins=[ar1_in[b][:]], outs=[ar1_out[b][:]])
                nc.gpsimd.dma_start(g1_sb[:, 2 * c0:2 * c1], ar1_out[b][:])

            def D1(b):
                """derive mu, rstd, packs for block b."""
                c0, c1 = b * BSEG, (b + 1) * BSEG
                ga = g1_sb[:, 2 * c0:2 * c0 + BSEG]
                gb = g1_sb[:, 2 * c0 + BSEG:2 * c1]
                nc.vector.tensor_scalar(mu_T[:, c0:c1], ga, rcnt, None,
                                        ALU.mult)
                nc.vector.tensor_scalar(es2[:, c0:c1], gb, rcnt, None,
                                        ALU.mult)
                nc.vector.tensor_tensor(var_T[:, c0:c1], mu_T[:, c0:c1],
                                        mu_T[:, c0:c1], ALU.mult)
                nc.vector.scalar_tensor_tensor(var_T[:, c0:c1],
                                               var_T[:, c0:c1], -1.0,
                                               es2[:, c0:c1],
                                               ALU.mult, ALU.add)
                nc.scalar.activation(sd_T[:, c0:c1], var_T[:, c0:c1],
                                     ACTF.Sqrt, bias=eps_col[:])
                nc.vector.reciprocal(rstd_T[:, c0:c1], sd_T[:, c0:c1])
                nc.vector.tensor_copy(mu_pack[0:64, c0:c1], mu_T[:, c0:c1])
                nc.scalar.dma_start(mu_pack[64:128, c0:c1],
                                  mu_pack[0:64, c0:c1])
                nc.vector.tensor_scalar(nmu_pack[0:64, c0:c1],
                                        mu_T[:, c0:c1], -1.0, None, ALU.mult)
                nc.scalar.dma_start(nmu_pack[64:128, c0:c1],
                                  nmu_pack[0:64, c0:c1])

            def S2(b):
                """pass 2 streaming for block b: sum(relu(x - mu))."""
                for gi in range(gpb):
                    g = b * gpb + gi
                    xt = get_group(g)
                    for k in range(GSEG):
                        s = g * GSEG + k
                        sk = s - b * BSEG
                        lo, hi = k * lp2, (k + 1) * lp2
                        if sk < P2_DVE:
                            # sum(max(x, mu)); corrected by -LP2*mu in C2F2
                            nc.vector.tensor_scalar(
                                sd_scr[:], xt[:, lo:hi],
                                mu_pack[:, s:s + 1], None,
                                ALU.max, op1=ALU.add,
                                accum_out=msC_T[:, s:s + 1])
                        else:
                            # relu(x - mu) summed directly on Act
                            nc.scalar.activation(
                                sa_scr[:], xt[:, lo:hi], ACTF.Relu,
                                bias=nmu_pack[:, s:s + 1],
                                accum_out=msC_T[:, s:s + 1])

            def C2F2(b):
                """correct DVE relu-sums, fold, AllReduce #2 for block b."""
                c0, c1 = b * BSEG, (b + 1) * BSEG
                nc.vector.scalar_tensor_tensor(
                    msC_T[:, c0:c0 + P2_DVE], mu_pack[:, c0:c0 + P2_DVE],
                    float(-lp2), msC_T[:, c0:c0 + P2_DVE],
                    ALU.mult, ALU.add)
                fps = psp.tile([64, BSEG], F32, tag="fold2")
                nc.tensor.matmul(fps[:], ii_sb[:], msC_T[:, c0:c1],
                                 start=True, stop=True, skip_group_check=True)
                nc.scalar.copy(fold2_sb[:, c0:c1], fps[:])
                nc.gpsimd.dma_start(ar2_in[b][:], fold2_sb[:, c0:c1])
                nc.gpsimd.collective_compute(
                    "AllReduce", ALU.add, replica_groups=rg,
                    ins=[ar2_in[b][:]], outs=[ar2_out[b][:]])
                nc.gpsimd.dma_start(g2_sb[:, c0:c1], ar2_out[b][:])

            def D2(b):
                """m, ECA conv, sigmoid, S3/B3 packs for block b."""
                c0, c1 = b * BSEG, (b + 1) * BSEG
                nc.vector.tensor_scalar(m_T[:, c0:c1], g2_sb[:, c0:c1],
                                        rcnt, None, ALU.mult)
                nc.vector.tensor_tensor(m_T[:, c0:c1], m_T[:, c0:c1],
                                        rstd_T[:, c0:c1], ALU.mult)
                cps = psp.tile([64, BSEG], F32, tag="conv")
                nc.tensor.matmul(cps[:], band_sb[:], m_T[:, c0:c1],
                                 start=True, stop=True, skip_group_check=True)
                nc.scalar.activation(w_T[:, c0:c1], cps[:], ACTF.Sigmoid)
                nc.vector.tensor_tensor(s3_T[:, c0:c1], rstd_T[:, c0:c1],
                                        w_T[:, c0:c1], ALU.mult)
                nc.vector.scalar_tensor_tensor(b3_T[:, c0:c1], mu_T[:, c0:c1],
                                               -1.0, s3_T[:, c0:c1],
                                               ALU.mult, ALU.mult)
                nc.vector.tensor_copy(scale_pack[0:64, c0:c1], s3_T[:, c0:c1])
                nc.scalar.dma_start(scale_pack[64:128, c0:c1],
                                  scale_pack[0:64, c0:c1])
                nc.vector.tensor_copy(bias_pack[0:64, c0:c1], b3_T[:, c0:c1])
                nc.scalar.dma_start(bias_pack[64:128, c0:c1],
                                  bias_pack[0:64, c0:c1])

            def S3(b):
                """pass 3 streaming for block b: out = relu(x*S3 + B3)."""
                for gi in range(gpb):
                    g = b * gpb + gi
                    xt = get_group(g)
                    ot = outp.tile([128, glen], F16, tag="o")
                    for k in range(GSEG):
                        s = g * GSEG + k
                        sk = s - b * BSEG
                        lo, hi = k * lp2, (k + 1) * lp2
                        if sk < P3_DVE:
                            nc.vector.tensor_scalar(
                                ot[:, lo:hi], xt[:, lo:hi],
                                scale_pack[:, s:s + 1],
                                bias_pack[:, s:s + 1], ALU.mult, op1=ALU.add)
                            nc.vector.tensor_scalar(
                                ot[:, lo:hi], ot[:, lo:hi], 0.0, None,
                                ALU.max)
                        else:
                            nc.scalar.activation(
                                ot[:, lo:hi], xt[:, lo:hi], ACTF.Relu,
                                bias=bias_pack[:, s:s + 1],
                                scale=scale_pack[:, s:s + 1])
                    nc.scalar.dma_start(out_ext[:, g * glen:(g + 1) * glen],
                                      ot[:])

            # ---- pipelined schedule over blocks ----
            S1(0); F1(0)
            S1(1); F1(1)
            S1(2); F1(2)
            D1(0); S2(0); C2F2(0)
            S1(3); F1(3)
            D1(1); S2(1); C2F2(1)
            D2(0); S3(0)
            D1(2); S2(2); C2F2(2)
            D2(1); S3(1)
            D1(3); S2(3); C2F2(3)
            D2(2); S3(2)
            D2(3); S3(3)

    nc.compile()
    return nc


_cache = {}


def _get_nc(lp2):
    if lp2 not in _cache:
        _cache[lp2] = build_nc(lp2)
    return _cache[lp2]


last_result = None


def _install_ntff_hook():
    """Provide antenv.axon_hooks (missing in this image) so
    run_bass_kernel_spmd(trace=True) can reach the axon NTFF profiler."""
    import types

    try:
        from antenv.axon_hooks import get_axon_ntff_profile_hook  # noqa: F401
        return
    except ImportError:
        pass
    if "/root/.axon_site" not in sys.path:
        sys.path.insert(0, "/root/.axon_site")
    from trn_agent_boot.trn_boot import _ntff_profile_via_ctypes
    hook = _ntff_profile_via_ctypes("/opt/axon/libaxon_pjrt.so")
    try:
        import antenv
    except ImportError:
        antenv = types.ModuleType("antenv")
        sys.modules["antenv"] = antenv
    mod = types.ModuleType("antenv.axon_hooks")
    mod.get_axon_ntff_profile_hook = lambda: hook
    mod.set_axon_ntff_profile_hook = lambda h: None
    sys.modules["antenv.axon_hooks"] = mod
    antenv.axon_hooks = mod
    import concourse.bass_utils as _bu
    _bu.upload_artifacts = lambda d: "local://" + str(d)


def _prep_inputs(x, idx, eca):
    """Sort by segment, deal evenly over cores, pad each (core, seg) run
    to the common even length LP with duplicated points."""
    order = np.argsort(idx, kind="stable")
    counts = np.bincount(idx, minlength=NSEG).astype(np.int64)
    starts = np.zeros(NSEG + 1, np.int64)
    starts[1:] = np.cumsum(counts)

    q, r = np.divmod(counts, NCORES)
    maxchunk = int((q + (r > 0).astype(np.int64)).max())
    lp = max(2, ((maxchunk + 1) // 2) * 2)       # even
    lp2 = lp // 2

    grids = []
    in_maps = []
    for kcore in range(NCORES):
        grid = np.empty((NSEG, lp), np.int64)
        for s in range(NSEG):
            n_s = counts[s]
            run = order[starts[s]:starts[s] + n_s]
            qq, rr = divmod(int(n_s), NCORES)
            a = kcore * qq + min(kcore, rr)
            b = a + qq + (1 if kcore < rr else 0)
            chunk = run[a:b]
            assert chunk.size > 0, f"empty (core,seg)=({kcore},{s})"
            grid[s] = np.resize(chunk, lp)
        grids.append(grid)
        xg = x[grid.reshape(-1)].reshape(NSEG, lp2, 2, C)
        dev = np.ascontiguousarray(
            xg.transpose(2, 3, 0, 1).reshape(128, NSEG * lp2),
            dtype=np.float16)
        in_maps.append({"xT": dev, "eca_weight": eca})
    return in_maps, grids, lp2


def kernel(features, ins_indices_batch, eca_weight, _trace=False):
    global last_result
    x = np.asarray(features, np.float32)
    idx = np.asarray(ins_indices_batch, np.int32)
    eca = np.asarray(eca_weight, np.float32).reshape(1, 3)
    n = x.shape[0]

    in_maps, grids, lp2 = _prep_inputs(x, idx, eca)
    nc = _get_nc(lp2)

    if _trace:
        _install_ntff_hook()
    try:
        res = run_bass_kernel_spmd(nc, in_maps, core_ids=list(range(NCORES)),
                                   trace=_trace)
    except Exception:
        if not _trace:
            raise
        import traceback
        traceback.print_exc()
        print("traced run failed; falling back to untraced", flush=True)
        res = run_bass_kernel_spmd(nc, in_maps, core_ids=list(range(NCORES)))
    last_result = res

    out = np.empty((n, C), np.float32)
    lp = 2 * lp2
    for kcore in range(NCORES):
        od = res.results[kcore]["out"]            # [128, NSEG*lp2] f16
        vals = od.reshape(2, C, NSEG, lp2).transpose(2, 3, 0, 1)
        out[grids[kcore].reshape(-1)] = vals.reshape(NSEG * lp, C)
    return out


if __name__ == "__main__":
    rng = np.random.default_rng(0)
    n_test = 200_000
    x = rng.standard_normal((n_test, C), dtype=np.float32)
    ii = rng.integers(0, NSEG, n_test).astype(np.int32)
    k = (rng.standard_normal((1, 1, 3)) * 0.1).astype(np.float32)
    out = kernel(x, ii, k)

    seg = ii
    cnt = np.maximum(np.bincount(seg, minlength=NSEG), 1).astype(np.float64)
    s = np.zeros((NSEG, C)); np.add.at(s, seg, x.astype(np.float64))
    s2 = np.zeros((NSEG, C)); np.add.at(s2, seg, x.astype(np.float64) ** 2)
    mu = s / cnt[:, None]
    var = s2 / cnt[:, None] - mu ** 2
    xn = (x - mu[seg]) / np.sqrt(var[seg] + EPS)
    xr = np.maximum(xn, 0)
    m = np.zeros((NSEG, C)); np.add.at(m, seg, xr)
    m = m / cnt[:, None]
    kf = k.reshape(3)
    mp = np.pad(m, ((0, 0), (1, 1)))
    conv = kf[0] * mp[:, 0:64] + kf[1] * mp[:, 1:65] + kf[2] * mp[:, 2:66]
    w = 1.0 / (1.0 + np.exp(-conv))
    exp = xr * w[seg]
    err = np.linalg.norm(out - exp) / np.linalg.norm(exp)
    print("out", out.shape, out.dtype, "rel_err", err)
